# revision 31
# baseline (speedup 1.0000x reference)
"""BitNet transformer block on 8 Trainium2 NeuronCores (Bass/Tile SPMD).

Sharding: data-parallel fold-balanced attention (core i owns query blocks
{i, 15-i} of 16 x 128 tokens; kv-proj token-sharded + AllGather), then
tensor-parallel MLP over INTER/8 with a token-chunked ReduceScatter.
Weights are ternary-quantized on host (exact {-1,0,+1} in bf16) with fp32
per-feature scales applied at PSUM evict. The residual after o_proj is
returned per-core and added during host-side unshard assembly.
"""

import sys

import numpy as np

try:
    import concourse.bass as bass  # noqa: F401
except Exception:  # pragma: no cover
    sys.path.insert(0, "/opt/trn_rl_repo")

import ml_dtypes
import concourse.bass as bass
import concourse.mybir as mybir
import concourse.tile as tile
from concourse import bacc
from concourse.bass_utils import run_bass_kernel_spmd

FP32 = mybir.dt.float32
BF16 = mybir.dt.bfloat16
F8 = mybir.dt.float8e4
BF = ml_dtypes.bfloat16
E4 = ml_dtypes.float8_e4m3

ALPHA = 0.7
EPS = 1e-5
NH = 16          # query heads
NKV = 4          # kv heads
D = 128          # head dim
H = 2048         # hidden
I_TOT = 8192     # mlp intermediate
S = 2048         # sequence
NC = 8           # cores
P = 128
HT = H // P      # 16 hidden tiles
B = S // P       # 16 token blocks
I_LOC = I_TOT // NC   # 1024 intermediate per core
IT = I_LOC // P       # 8 inter tiles per core
TOK = 256             # tokens owned per core (2 blocks)
NCHUNK = 4            # reduce-scatter token chunks
CH = S // NCHUNK      # 512 tokens per chunk

# gathered token order: core i contributes blocks [i, 15-i]
PERM = []
for _i in range(NC):
    PERM += [_i, 15 - _i]
# MLP/RS token order: lo blocks 0..7 then hi blocks 15..8
PERM_DOWN = list(range(8)) + list(range(15, 7, -1))

_CACHE = {}


def _build_program():
    nc = bacc.Bacc("TRN2", target_bir_lowering=False, debug=False, num_devices=NC)
    AF = mybir.ActivationFunctionType
    ALU = mybir.AluOpType
    rg = [list(range(NC))]
    CH = 256          # MLP token chunk (= one rank's tokens)
    NCH = 8
    HH = HT // 2

    # ---------------- inputs ----------------
    def dram_in(name, shape, dt=FP32):
        return nc.dram_tensor(name, shape, dt, kind="ExternalInput")

    xT_f = dram_in("xT_f", [P, HT, S], BF16)          # bf16 x^T ALL tokens (ln1 only)
    xT_own = dram_in("xT_own", [P, HT, TOK])          # fp32 x^T own cols (residual)
    cos_f = dram_in("cos_f", [P, S])
    sin_f = dram_in("sin_f", [P, S])
    wq_in = dram_in("wq", [P, 2, HT, P], BF16)        # my 2 heads [p, f, kt, c]
    wk_in = dram_in("wk", [P, HT, P], BF16)           # my kv head
    wv_in = dram_in("wv", [P, HT, P], BF16)           # my kv head (lhsT like k)
    wo_in = dram_in("wo", [HT, P, HT, P], BF16)
    wg_in = dram_in("wg", [IT, P, HT, P], F8)
    wu_in = dram_in("wu", [IT, P, HT, P], F8)
    wd_in = dram_in("wd", [P, IT, H], F8)           # [p, it, ho]
    # packed constants: one fp32 blob + one bf16 blob (single DMA each)
    cb32_in = dram_in("cb32", [P, 181])
    cb16_in = dram_in("cb16", [P, 513], BF16)

    outT = nc.dram_tensor("outT", [TOK, H], BF16, kind="ExternalOutput")
    xmidT = nc.dram_tensor("xmidT", [P, HT, TOK], FP32, kind="ExternalOutput")

    a2a_lo_in = nc.dram_tensor("a2a_lo_in", [NC, P, 2, P], BF16)
    a2a_lo_out = nc.dram_tensor("a2a_lo_out", [NC, P, 2, P], BF16)
    a2a_hi_in = nc.dram_tensor("a2a_hi_in", [NC, P, 2, P], BF16)
    a2a_hi_out = nc.dram_tensor("a2a_hi_out", [NC, P, 2, P], BF16)
    h2_in_lo = nc.dram_tensor("h2_in_lo", [P, HT, P], F8)
    h2_in_hi = nc.dram_tensor("h2_in_hi", [P, HT, P], F8)
    h2_glo = nc.dram_tensor("h2_glo", [NC * P, HT, P], F8, addr_space="Shared")
    h2_ghi = nc.dram_tensor("h2_ghi", [NC * P, HT, P], F8, addr_space="Shared")

    with tile.TileContext(nc) as tc:
        const = tc.alloc_tile_pool(name="const", bufs=1)
        cb32 = const.tile([P, 181], FP32)
        cb16 = const.tile([P, 513], BF16)
        nc.sync.dma_start(cb32[:], cb32_in[:])
        nc.sync.dma_start(cb16[:], cb16_in[:])
        ones_f = cb32[:, 0:128]
        aq = cb32[:, 128:130]
        ak = cb32[:, 130:131]
        av = cb32[:, 131:132]
        ao = cb32[:, 132:148]
        ag = cb32[:, 148:156]
        au = cb32[:, 156:164]
        ad = cb32[:, 164:180]
        eps_t = cb32[:, 180:181]
        ones_b = cb16[:, 0:1]
        rT = cb16[:, 1:129]
        iden = cb16[:, 129:257]
        tril2 = cb16[:, 257:513]

        wdres = tc.alloc_tile_pool(name="wd_res", bufs=1)
        wd_sb = wdres.tile([P, IT, H], F8)
        nc.scalar.dma_start(wd_sb[:], wd_in[:])
        wgur = tc.alloc_tile_pool(name="wgu_res", bufs=1)
        wg_sb = wgur.tile([P, IT, HT, P], F8)
        wu_sb = wgur.tile([P, IT, HT, P], F8)
        for _f in range(IT):
            nc.scalar.dma_start(wg_sb[:, _f], wg_in[_f])
            nc.scalar.dma_start(wu_sb[:, _f], wu_in[_f])

        midpool = tc.alloc_tile_pool(name="midpool", bufs=1)
        x_mid = midpool.tile([P, HT, TOK], FP32)
        xopool = tc.alloc_tile_pool(name="xopool", bufs=1)
        xo = xopool.tile([P, HT, TOK], FP32)
        omypool = tc.alloc_tile_pool(name="omypool", bufs=1)
        o_my = omypool.tile([P, HT, TOK], BF16)      # post-A2A: 16 heads x my toks
        qkvpool = tc.alloc_tile_pool(name="qkvpool", bufs=1)
        q_my = qkvpool.tile([P, 2, S], BF16)         # my 2 heads, all tokens
        k_my = qkvpool.tile([P, B, P], BF16)         # my kv head [d, blk, tok]
        v_my = qkvpool.tile([P, B, P], BF16)         # my kv head [tok, blk, d]

        def rms_bcast(src3d, nt, psp, tmp):
            """[P,HT,nt] -> [P,nt] fp32 SBUF broadcast of 1/sqrt(mean_sq+eps).

            Squares on ScalarE, kt-tree-sum on GpSimd (both otherwise idle),
            one ones-mm partition-reduce, Abs_reciprocal_sqrt, PE broadcast."""
            ssq = psp.tile([1, nt], FP32, name="ssq", tag="rmsps")
            for kt in range(HT):
                sqv = tmp.tile([P, nt], BF16, name="sqv")
                nc.scalar.activation(sqv[:], src3d[:, kt, :], AF.Square)
                nc.tensor.matmul(ssq[:], ones_b, sqv[:],
                                 start=(kt == 0), stop=(kt == HT - 1))
            rsq = tmp.tile([1, nt], FP32, name="rsq")
            nc.scalar.activation(rsq[:], ssq[:], AF.Abs_reciprocal_sqrt,
                                 bias=eps_t[0:1, :], scale=1.0 / H)
            bc = psp.tile([P, nt], FP32, name="bc", tag="rmsps")
            nc.tensor.matmul(bc[:], ones_f[0:1, :], rsq[:], start=True, stop=True)
            bcs = tmp.tile([P, nt], FP32, name="bcs")
            nc.scalar.activation(bcs[:], bc[:], AF.Copy)
            return bcs

        def rmsnorm_t(src3d, out3d, nt, psp, tmp):
            """[P,HT,nt] fp32 -> bf16 rmsnorm (partition reduce via ones-mm)"""
            bcs = rms_bcast(src3d, nt, psp, tmp)
            for kt in range(HT):
                nc.vector.tensor_mul(out3d[:, kt, :], src3d[:, kt, :], bcs[:])

        # ====== phase 1: ln1 (all tokens, chunked) + q/k/v TP projections ======
        CH4 = 512
        with tc.tile_pool(name="xc_pool", bufs=2) as xcp, \
             tc.tile_pool(name="hc_pool", bufs=2) as hcp, \
             tc.tile_pool(name="p1sb", bufs=2) as p1sb, \
             tc.tile_pool(name="p1ps", bufs=1, space="PSUM") as p1ps, \
             tc.tile_pool(name="wpool", bufs=2) as wp, \
             tc.tile_pool(name="p2ps", bufs=3, space="PSUM") as p2ps, \
             tc.tile_pool(name="rot_ps", bufs=2, space="PSUM") as rot_ps, \
             tc.tile_pool(name="vt_ps", bufs=2, space="PSUM") as vt_ps, \
             tc.tile_pool(name="p2sb", bufs=2) as p2sb, \
             tc.tile_pool(name="cs_pool", bufs=2) as csp, \
             tc.tile_pool(name="wres", bufs=1) as wres:
            wq_sb = wres.tile([P, 2, HT, P], BF16)
            nc.sync.dma_start(wq_sb[:], wq_in[:])
            wk_sb = wres.tile([P, HT, P], BF16)
            nc.sync.dma_start(wk_sb[:], wk_in[:])
            wv_sb = wres.tile([P, HT, P], BF16)
            nc.sync.dma_start(wv_sb[:], wv_in[:])
            for c4 in range(4):
                tsl = slice(c4 * CH4, (c4 + 1) * CH4)
                xc = xcp.tile([P, HT, CH4], BF16, name="xc")
                nc.sync.dma_start(xc[:], xT_f[:, :, tsl])
                cfc = csp.tile([P, CH4], FP32, name="cfc")
                nc.sync.dma_start(cfc[:], cos_f[:, tsl])
                sfc = csp.tile([P, CH4], FP32, name="sfc")
                nc.sync.dma_start(sfc[:], sin_f[:, tsl])
                with nc.named_scope("p1"):
                    # rmsnorm per-token scale s_t, folded into cos/sin and the
                    # v evict (projections are linear in s_t) — no normalized
                    # h tile is ever materialized.
                    bcs = rms_bcast(xc, CH4, p1ps, p1sb)
                    cf2 = csp.tile([P, CH4], BF16, name="cf2")
                    nc.vector.tensor_mul(cf2[:], cfc[:], bcs[:])
                    sf2 = csp.tile([P, CH4], BF16, name="sf2")
                    nc.vector.tensor_mul(sf2[:], sfc[:], bcs[:])
                    # q: my 2 heads
                    for f in range(2):
                        ps = p2ps.tile([P, CH4], FP32, name="pps")
                        for kt in range(HT):
                            nc.tensor.matmul(ps[:], wq_sb[:, f, kt, :],
                                             xc[:, kt, :],
                                             start=(kt == 0), stop=(kt == HT - 1))
                        qs = p2sb.tile([P, CH4], BF16, name="qs")
                        nc.vector.tensor_scalar_mul(qs[:], ps[:], aq[:, f:f + 1])
                        rot = rot_ps.tile([P, CH4], FP32, name="rot")
                        nc.tensor.matmul(rot[:], rT[:], qs[:], start=True, stop=True)
                        t1 = p2sb.tile([P, CH4], BF16, name="t1")
                        nc.vector.tensor_mul(t1[:], rot[:], sf2[:])
                        t2 = p2sb.tile([P, CH4], BF16, name="t2")
                        nc.vector.tensor_mul(t2[:], qs[:], cf2[:])
                        nc.vector.tensor_add(q_my[:, f, tsl], t1[:], t2[:])
                    # k: my kv head
                    ps = p2ps.tile([P, CH4], FP32, name="pps")
                    for kt in range(HT):
                        nc.tensor.matmul(ps[:], wk_sb[:, kt, :], xc[:, kt, :],
                                         start=(kt == 0), stop=(kt == HT - 1))
                    ks = p2sb.tile([P, CH4], BF16, name="qs")
                    nc.vector.tensor_scalar_mul(ks[:], ps[:], ak[:, 0:1])
                    rot = rot_ps.tile([P, CH4], FP32, name="rot")
                    nc.tensor.matmul(rot[:], rT[:], ks[:], start=True, stop=True)
                    t1 = p2sb.tile([P, CH4], BF16, name="t1")
                    nc.vector.tensor_mul(t1[:], rot[:], sf2[:])
                    t2 = p2sb.tile([P, CH4], BF16, name="t2")
                    nc.vector.tensor_mul(t2[:], ks[:], cf2[:])
                    nc.vector.tensor_add(
                        k_my[:, 4 * c4:4 * c4 + 4, :].rearrange("p b t -> p (b t)"),
                        t1[:], t2[:])
                    # v: my kv head (rms scale via stt), PE-transpose to [tok, d]
                    ps = p2ps.tile([P, CH4], FP32, name="pps")
                    for kt in range(HT):
                        nc.tensor.matmul(ps[:], wv_sb[:, kt, :], xc[:, kt, :],
                                         start=(kt == 0), stop=(kt == HT - 1))
                    vtv = p2sb.tile([P, CH4], BF16, name="vtv")
                    nc.vector.scalar_tensor_tensor(
                        vtv[:], ps[:], av[:, 0:1], bcs[:], ALU.mult, ALU.mult)
                    for j in range(4):
                        vtp = vt_ps.tile([P, P], BF16, name="vtp")
                        nc.tensor.transpose(vtp[:], vtv[:, j * P:(j + 1) * P],
                                            iden[:])
                        nc.vector.tensor_copy(v_my[:, 4 * c4 + j, :], vtp[:])

        # ============= phase 2: attention (triangle, paired heads) =============
        # exp batched over groups of 4 key-blocks (one ACT op per group).
        with tc.tile_pool(name="a_ps", bufs=2, space="PSUM") as a_ps, \
             tc.tile_pool(name="o_ps", bufs=2, space="PSUM") as o_ps, \
             tc.tile_pool(name="l_ps", bufs=1, space="PSUM") as l_ps, \
             tc.tile_pool(name="bc_ps", bufs=1, space="PSUM") as bc_ps, \
             tc.tile_pool(name="a_sb", bufs=3) as a_sb:
            for qb in range(B):
                r_dst = min(qb, 15 - qb)
                ops = o_ps.tile([P, TOK], FP32, name="ops")
                lps = l_ps.tile([1, TOK], FP32, name="lps")
                qv = q_my[:, :, qb * P:(qb + 1) * P]    # [P, 2, 128]
                with nc.named_scope("attn"):
                    ngrp = (qb + 4) // 4
                    for g in range(ngrp):
                        gsz = min(4, qb + 1 - 4 * g)
                        sps = a_ps.tile([P, 4 * TOK], FP32, name="sps")
                        for j in range(gsz):
                            nc.tensor.matmul(
                                sps[:, j * TOK:(j + 1) * TOK],
                                k_my[:, 4 * g + j, :], qv,
                                start=True, stop=True)
                        pm = a_sb.tile([P, 4 * TOK], BF16, name="pm")
                        nc.scalar.activation(pm[:, :gsz * TOK],
                                             sps[:, :gsz * TOK], AF.Exp)
                        for j in range(gsz):
                            kb = 4 * g + j
                            blk = pm[:, j * TOK:(j + 1) * TOK]
                            if kb == qb:
                                pmm = a_sb.tile([P, TOK], BF16, name="pmm")
                                nc.vector.tensor_mul(pmm[:], blk, tril2[:])
                                blk = pmm[:]
                            nc.tensor.matmul(lps[:], ones_b[:], blk,
                                             start=(kb == 0), stop=(kb == qb))
                            nc.tensor.matmul(ops[:], v_my[:, kb, :], blk,
                                             start=(kb == 0), stop=(kb == qb))
                    lsb = a_sb.tile([1, TOK], FP32, name="lsb")
                    nc.scalar.activation(lsb[:], lps[:], AF.Copy)
                    linv = a_sb.tile([1, TOK], FP32, name="linv")
                    nc.vector.reciprocal_approx_fast(linv[:], lsb[:])
                    bca = bc_ps.tile([P, TOK], FP32, name="bca")
                    nc.tensor.matmul(bca[:], ones_f[0:1, :], linv[:],
                                     start=True, stop=True)
                    bcs = a_sb.tile([P, TOK], FP32, name="bcs")
                    nc.scalar.activation(bcs[:], bca[:], AF.Copy)
                    osb = a_sb.tile([P, TOK], BF16, name="osb")
                    nc.vector.tensor_mul(osb[:], ops[:], bcs[:])
                dst = a2a_lo_in if qb < 8 else a2a_hi_in
                nc.sync.dma_start(
                    dst[r_dst][:],
                    osb[:].rearrange("p (h t) -> p h t", h=2))
                if qb == 7:
                    nc.gpsimd.collective_compute(
                        "AllToAll", ALU.bypass, ins=[a2a_lo_in[:]],
                        outs=[a2a_lo_out[:]], replica_groups=rg)
                    for hh in range(2):
                        nc.sync.dma_start(
                            o_my[:].rearrange("p (j h) t -> h p j t", h=2)[hh][:, :, 0:P],
                            a2a_lo_out[:].rearrange("j p h t -> h p j t")[hh])
            nc.gpsimd.collective_compute(
                "AllToAll", ALU.bypass, ins=[a2a_hi_in[:]],
                outs=[a2a_hi_out[:]], replica_groups=rg)
            for hh in range(2):
                nc.sync.dma_start(
                    o_my[:].rearrange("p (j h) t -> h p j t", h=2)[hh][:, :, P:TOK],
                    a2a_hi_out[:].rearrange("j p h t -> h p j t")[hh])
        qkvpool.release()

        # ============= phase 3: o_proj + residual + ln2 (token halves) =============
        with tc.tile_pool(name="wo_res", bufs=1) as wores, \
             tc.tile_pool(name="p5ps", bufs=2, space="PSUM") as p5ps, \
             tc.tile_pool(name="p5sb", bufs=3) as p5sb:
            nc.sync.dma_start(xo[:], xT_own[:])
            wo_all = wores.tile([P, HT, HT, P], BF16)
            for f in range(HT):
                nc.scalar.dma_start(wo_all[:, f, :, :], wo_in[f])
            for half, (h2_in, h2_g) in enumerate(
                    ((h2_in_lo, h2_glo), (h2_in_hi, h2_ghi))):
                csl = slice(half * P, (half + 1) * P)
                with nc.named_scope("oproj"):
                    for f in range(HT):
                        ps = p5ps.tile([P, P], FP32, name="ops5")
                        for kt in range(HT):
                            nc.tensor.matmul(ps[:], wo_all[:, f, kt, :],
                                             o_my[:, kt, csl],
                                             start=(kt == 0), stop=(kt == HT - 1))
                        nc.vector.scalar_tensor_tensor(
                            x_mid[:, f, csl], ps[:], ao[:, f:f + 1],
                            xo[:, f, csl], ALU.mult, ALU.add)
                    h2h = p5sb.tile([P, HT, P], F8, name="h2h", tag="h2h")
                    rmsnorm_t(x_mid[:, :, csl], h2h, P, p5ps, p5sb)
                nc.scalar.dma_start(h2_in[:], h2h[:])
                nc.gpsimd.collective_compute(
                    "AllGather", ALU.bypass, ins=[h2_in[:]],
                    outs=[h2_g[:]], replica_groups=rg)
            nc.gpsimd.dma_start(xmidT[:], x_mid[:])
        omypool.release()
        xopool.release()
        midpool.release()
        h2lov = h2_glo[:].rearrange("(r p) kt t -> r p kt t", r=NC)
        h2hiv = h2_ghi[:].rearrange("(r p) kt t -> r p kt t", r=NC)

        # ============= phase 5: MLP (TP over inter) + RS =============
        CHM = 512
        with tc.tile_pool(name="h2c_pool", bufs=2) as h2cp, \
             tc.tile_pool(name="m_pool", bufs=2) as mp, \
             tc.tile_pool(name="p7gu", bufs=3, space="PSUM") as p7gu, \
             tc.tile_pool(name="p7d", bufs=2, space="PSUM") as p7d, \
             tc.tile_pool(name="p7sb", bufs=3) as p7sb:
            for c in range(4):
                h2v = h2lov if c < 2 else h2hiv
                rbase = (c % 2) * 4
                h2c = h2cp.tile([P, HT, CHM], F8, name="h2c")
                for j in range(4):
                    nc.sync.dma_start(h2c[:, :, j * P:(j + 1) * P],
                                      h2v[rbase + j])
                m_all = mp.tile([P, IT, CHM], F8, name="m_all")
                with nc.named_scope("mlp"):
                    for f in range(IT):
                        gps = p7gu.tile([P, CHM], FP32, name="gps")
                        for tp in range(HT // 2):
                            nc.tensor.matmul(
                                gps[:], wg_sb[:, f, 2 * tp:2 * tp + 2, :],
                                h2c[:, 2 * tp:2 * tp + 2, :],
                                start=(tp == 0), stop=(tp == HT // 2 - 1),
                                perf_mode=mybir.MatmulPerfMode.DoubleRow)
                        ups = p7gu.tile([P, CHM], FP32, name="ups")
                        for tp in range(HT // 2):
                            nc.tensor.matmul(
                                ups[:], wu_sb[:, f, 2 * tp:2 * tp + 2, :],
                                h2c[:, 2 * tp:2 * tp + 2, :],
                                start=(tp == 0), stop=(tp == HT // 2 - 1),
                                perf_mode=mybir.MatmulPerfMode.DoubleRow)
                        # relu(ag*g) then square on ScalarE; final mul on DVE
                        grl = p7sb.tile([P, CHM], BF16, name="grl")
                        nc.scalar.activation(grl[:], gps[:], AF.Relu,
                                             scale=ag[:, f:f + 1])
                        g2 = p7sb.tile([P, CHM], BF16, name="g2")
                        nc.scalar.activation(g2[:], grl[:], AF.Square)
                        nc.vector.scalar_tensor_tensor(m_all[:, f, :], ups[:],
                                                       au[:, f:f + 1], g2[:],
                                                       ALU.mult, ALU.mult)
                rs_a = nc.dram_tensor(f"rs_in_{c}a", [H, TOK], BF16)
                rs_b = nc.dram_tensor(f"rs_in_{c}b", [H, TOK], BF16)
                rs_iva = rs_a[:].rearrange("(f p) t -> f p t", p=P)
                rs_ivb = rs_b[:].rearrange("(f p) t -> f p t", p=P)
                if c < 3:
                    for f in range(HT):
                        dps = p7d.tile([P, CHM], FP32, name="dps")
                        for tp in range(IT // 2):
                            nc.tensor.matmul(
                                dps[:], wd_sb[:, 2 * tp:2 * tp + 2, f * P:(f + 1) * P],
                                m_all[:, 2 * tp:2 * tp + 2, :],
                                start=(tp == 0), stop=(tp == IT // 2 - 1),
                                perf_mode=mybir.MatmulPerfMode.DoubleRow)
                        dn = p7sb.tile([P, CHM], BF16, name="dn")
                        nc.vector.tensor_scalar_mul(dn[:], dps[:], ad[:, f:f + 1])
                        nc.sync.dma_start(rs_iva[f], dn[:, 0:TOK])
                        nc.sync.dma_start(rs_ivb[f], dn[:, TOK:CHM])
                    for hf, rs_in in enumerate((rs_a, rs_b)):
                        rs_out = nc.dram_tensor(f"rso_{c}{hf}", [TOK, TOK], BF16)
                        nc.gpsimd.collective_compute(
                            "ReduceScatter", ALU.add, ins=[rs_in[:]],
                            outs=[rs_out[:]], replica_groups=rg)
                        nc.sync.dma_start(
                            outT[:, (2 * c + hf) * TOK:(2 * c + hf + 1) * TOK],
                            rs_out[:])
                else:
                    # last chunk: split down by token halves so RS pieces overlap
                    for hf, (rs_in, rs_iv) in enumerate(
                            ((rs_a, rs_iva), (rs_b, rs_ivb))):
                        tsl2 = slice(hf * TOK, (hf + 1) * TOK)
                        for f in range(HT):
                            dps = p7d.tile([P, CHM], FP32, name="dps")[:, 0:TOK]
                            for tp in range(IT // 2):
                                nc.tensor.matmul(
                                    dps[:],
                                    wd_sb[:, 2 * tp:2 * tp + 2, f * P:(f + 1) * P],
                                    m_all[:, 2 * tp:2 * tp + 2, tsl2],
                                    start=(tp == 0), stop=(tp == IT // 2 - 1),
                                    perf_mode=mybir.MatmulPerfMode.DoubleRow)
                            dn = p7sb.tile([P, CHM], BF16, name="dn")[:, 0:TOK]
                            nc.vector.tensor_scalar_mul(dn[:], dps[:],
                                                        ad[:, f:f + 1])
                            nc.sync.dma_start(rs_iv[f], dn[:])
                        rs_out = nc.dram_tensor(f"rso_3{hf}", [TOK, TOK], BF16)
                        nc.gpsimd.collective_compute(
                            "ReduceScatter", ALU.add, ins=[rs_in[:]],
                            outs=[rs_out[:]], replica_groups=rg)
                        nc.sync.dma_start(
                            outT[:, (6 + hf) * TOK:(7 + hf) * TOK], rs_out[:])

        wgur.release()
        wdres.release()
        const.release()

    nc.finalize()
    return nc


def _ternary(w, fold_row=None):
    """Quantize [O, Hin] fp32 -> (ternary fp32 {-1,0,1}, absmean [O])."""
    w = np.asarray(w, dtype=np.float32)
    am = np.mean(np.abs(w), axis=1)
    t = np.sign(w) * (np.abs(w) > ALPHA * am[:, None]).astype(np.float32)
    if fold_row is not None:
        t = t * fold_row[None, :]
    return t, am


def _wlhsT(tern, n_f):
    """ternary [O, Hin] -> lhsT input layout [f, p, kt, c] bf16 (tile (kt,f):
    rows Hin-chunk kt, cols O-chunk f)."""
    o, hin = tern.shape
    kt = hin // P
    assert n_f * P == o
    wT = np.ascontiguousarray(tern.T)  # [Hin, O]
    return np.ascontiguousarray(
        wT.reshape(kt, P, n_f, P).transpose(2, 1, 0, 3)).astype(BF)


def _scale_tiles(a):
    """[O] -> [P, O//P] with column f = features f*128..f*128+127."""
    return np.ascontiguousarray(a.reshape(-1, P).T).astype(np.float32)


def _pcol(x2d):
    """[K, T] -> [P, K//P, T] (partition-major for direct DMA)."""
    k, t = x2d.shape
    return np.ascontiguousarray(
        x2d.reshape(k // P, P, t).transpose(1, 0, 2)).astype(np.float32)


def kernel(x, cos, sin, wq, wk, wv, wo, wg, wu, wd, ln1_w, ln2_w):
    x = np.asarray(x, dtype=np.float32)
    b, s, hdim = x.shape
    assert (b, s, hdim) == (1, S, H)

    if "nc" not in _CACHE:
        _CACHE["nc"] = _build_program()
    nc = _CACHE["nc"]

    ln1 = np.asarray(ln1_w, dtype=np.float32)
    ln2 = np.asarray(ln2_w, dtype=np.float32)

    tq, amq = _ternary(wq, fold_row=ln1)
    tk, amk = _ternary(wk, fold_row=ln1)
    tv, amv = _ternary(wv, fold_row=ln1)
    to, amo = _ternary(wo)
    tg, amg = _ternary(wg, fold_row=ln2)
    tu, amu = _ternary(wu, fold_row=ln2)
    td, amd = _ternary(wd)

    wq_h = _wlhsT(tq, NH)        # [16, P, HT, P]
    wk_h = _wlhsT(tk, NKV)       # [4, P, HT, P]
    wv_h = _wlhsT(tv, NKV)
    wo_h = _wlhsT(to, HT)
    wg_h = _wlhsT(tg, I_TOT // P)
    wu_h = _wlhsT(tu, I_TOT // P)
    wd_h = np.ascontiguousarray(
        td.T.reshape(I_TOT // P, P, H).transpose(1, 0, 2)).astype(BF)  # [P,64,H]

    aq_h = _scale_tiles(amq / np.sqrt(np.float32(D)))
    ak_h = _scale_tiles(amk)
    av_h = _scale_tiles(amv)
    ao_h = _scale_tiles(amo)
    ag_h = _scale_tiles(amg)
    au_h = _scale_tiles(amu)
    ad_h = _scale_tiles(amd)

    x2 = x[0]
    xT = np.ascontiguousarray(x2.T)
    xT_f = _pcol(xT)
    cosT = np.ascontiguousarray(np.asarray(cos, np.float32)[0, 0].T)
    sinT = np.ascontiguousarray(np.asarray(sin, np.float32)[0, 0].T)

    R = np.zeros((P, P), np.float32)
    for m in range(64):
        R[m, m + 64] = -1.0
        R[m + 64, m] = 1.0
    rT_h = np.ascontiguousarray(R.T).astype(BF)
    ones_f = np.ones((P, P), np.float32)
    ones_b = np.ones((P, 1), np.float32).astype(BF)
    triu = np.triu(np.ones((P, P), np.float32))
    tril2_h = np.ascontiguousarray(np.concatenate([triu, triu], axis=1)).astype(BF)
    iden_h = np.eye(P, dtype=np.float32).astype(BF)

    in_maps = []
    for i in range(NC):
        blo, bhi = i, 15 - i
        own_cols = np.r_[blo * P:(blo + 1) * P, bhi * P:(bhi + 1) * P]
        kvh = i // 2
        islice = slice(i * IT, (i + 1) * IT)
        cb32 = np.ascontiguousarray(np.concatenate([
            ones_f,
            aq_h[:, 2 * i:2 * i + 2],
            ak_h[:, kvh:kvh + 1],
            av_h[:, kvh:kvh + 1],
            ao_h,
            ag_h[:, islice],
            au_h[:, islice],
            ad_h,
            np.full((P, 1), EPS, np.float32),
        ], axis=1)).astype(np.float32)
        cb16 = np.ascontiguousarray(np.concatenate([
            ones_b.astype(np.float32), rT_h.astype(np.float32),
            iden_h.astype(np.float32), tril2_h.astype(np.float32),
        ], axis=1)).astype(BF)
        in_maps.append({
            "xT_f": xT_f.astype(BF),
            "xT_own": _pcol(xT[:, own_cols]),
            "cos_f": cosT, "sin_f": sinT,
            "wq": np.ascontiguousarray(wq_h[2 * i:2 * i + 2].transpose(1, 0, 2, 3)),
            "wk": np.ascontiguousarray(wk_h[kvh]),
            "wv": np.ascontiguousarray(wv_h[kvh]),
            "wo": wo_h,
            "wg": np.ascontiguousarray(wg_h[islice]).astype(E4),
            "wu": np.ascontiguousarray(wu_h[islice]).astype(E4),
            "wd": np.ascontiguousarray(wd_h[:, islice, :]).astype(E4),
            "cb32": cb32, "cb16": cb16,
        })

    res = run_bass_kernel_spmd(nc, in_maps, list(range(NC)))
    _CACHE["last_result"] = res

    down_T = np.concatenate(
        [res.results[i]["outT"].astype(np.float32) for i in range(NC)], axis=0)
    xmid_T = np.concatenate(
        [res.results[i]["xmidT"].transpose(1, 0, 2).reshape(H, TOK)
         for i in range(NC)], axis=1)
    out_T = np.empty_like(down_T)
    for j, blk in enumerate(PERM_DOWN):
        out_T[:, blk * P:(blk + 1) * P] = down_T[:, j * P:(j + 1) * P]
    for j, blk in enumerate(PERM):
        out_T[:, blk * P:(blk + 1) * P] += xmid_T[:, j * P:(j + 1) * P]
    return np.ascontiguousarray(out_T.T).reshape(1, S, H).astype(np.float32)


if __name__ == "__main__":
    nc = _build_program()
    print("build OK; instructions:",
          sum(len(b.instructions) for f in nc.m.functions for b in f.blocks))



# revision 32
# speedup vs baseline: 1.0085x; 1.0085x over previous
"""BitNet transformer block on 8 Trainium2 NeuronCores (Bass/Tile SPMD).

Sharding: data-parallel fold-balanced attention (core i owns query blocks
{i, 15-i} of 16 x 128 tokens; kv-proj token-sharded + AllGather), then
tensor-parallel MLP over INTER/8 with a token-chunked ReduceScatter.
Weights are ternary-quantized on host (exact {-1,0,+1} in bf16) with fp32
per-feature scales applied at PSUM evict. The residual after o_proj is
returned per-core and added during host-side unshard assembly.
"""

import sys

import numpy as np

try:
    import concourse.bass as bass  # noqa: F401
except Exception:  # pragma: no cover
    sys.path.insert(0, "/opt/trn_rl_repo")

import ml_dtypes
import concourse.bass as bass
import concourse.mybir as mybir
import concourse.tile as tile
from concourse import bacc
from concourse.bass_utils import run_bass_kernel_spmd

FP32 = mybir.dt.float32
BF16 = mybir.dt.bfloat16
F8 = mybir.dt.float8e4
BF = ml_dtypes.bfloat16
E4 = ml_dtypes.float8_e4m3

ALPHA = 0.7
EPS = 1e-5
NH = 16          # query heads
NKV = 4          # kv heads
D = 128          # head dim
H = 2048         # hidden
I_TOT = 8192     # mlp intermediate
S = 2048         # sequence
NC = 8           # cores
P = 128
HT = H // P      # 16 hidden tiles
B = S // P       # 16 token blocks
I_LOC = I_TOT // NC   # 1024 intermediate per core
IT = I_LOC // P       # 8 inter tiles per core
TOK = 256             # tokens owned per core (2 blocks)
NCHUNK = 4            # reduce-scatter token chunks
CH = S // NCHUNK      # 512 tokens per chunk

# gathered token order: core i contributes blocks [i, 15-i]
PERM = []
for _i in range(NC):
    PERM += [_i, 15 - _i]
# MLP/RS token order: lo blocks 0..7 then hi blocks 15..8
PERM_DOWN = list(range(8)) + list(range(15, 7, -1))

_CACHE = {}


def _build_program():
    nc = bacc.Bacc("TRN2", target_bir_lowering=False, debug=False, num_devices=NC)
    AF = mybir.ActivationFunctionType
    ALU = mybir.AluOpType
    rg = [list(range(NC))]
    CH = 256          # MLP token chunk (= one rank's tokens)
    NCH = 8
    HH = HT // 2

    # ---------------- inputs ----------------
    def dram_in(name, shape, dt=FP32):
        return nc.dram_tensor(name, shape, dt, kind="ExternalInput")

    xT_f = dram_in("xT_f", [P, HT, S], BF16)          # bf16 x^T ALL tokens (ln1 only)
    xT_own = dram_in("xT_own", [P, HT, TOK])          # fp32 x^T own cols (residual)
    cos_f = dram_in("cos_f", [P, S])
    sin_f = dram_in("sin_f", [P, S])
    wq_in = dram_in("wq", [P, 2, HT, P], BF16)        # my 2 heads [p, f, kt, c]
    wk_in = dram_in("wk", [P, HT, P], BF16)           # my kv head
    wv_in = dram_in("wv", [P, HT, P], BF16)           # my kv head (lhsT like k)
    wo_in = dram_in("wo", [HT, P, HT, P], BF16)
    wg_in = dram_in("wg", [IT, P, HT, P], F8)
    wu_in = dram_in("wu", [IT, P, HT, P], F8)
    wd_in = dram_in("wd", [P, IT, H], F8)           # [p, it, ho]
    # packed constants: one fp32 blob + one bf16 blob (single DMA each)
    cb32_in = dram_in("cb32", [P, 181])
    cb16_in = dram_in("cb16", [P, 513], BF16)

    outT = nc.dram_tensor("outT", [TOK, H], BF16, kind="ExternalOutput")
    xmidT = nc.dram_tensor("xmidT", [P, HT, TOK], FP32, kind="ExternalOutput")

    a2a_lo_in = nc.dram_tensor("a2a_lo_in", [NC, P, 2, P], BF16)
    a2a_lo_out = nc.dram_tensor("a2a_lo_out", [NC, P, 2, P], BF16)
    a2a_hi_in = nc.dram_tensor("a2a_hi_in", [NC, P, 2, P], BF16)
    a2a_hi_out = nc.dram_tensor("a2a_hi_out", [NC, P, 2, P], BF16)
    h2_in_lo = nc.dram_tensor("h2_in_lo", [P, HT, P], F8)
    h2_in_hi = nc.dram_tensor("h2_in_hi", [P, HT, P], F8)
    h2_glo = nc.dram_tensor("h2_glo", [NC * P, HT, P], F8, addr_space="Shared")
    h2_ghi = nc.dram_tensor("h2_ghi", [NC * P, HT, P], F8, addr_space="Shared")

    with tile.TileContext(nc) as tc:
        const = tc.alloc_tile_pool(name="const", bufs=1)
        cb32 = const.tile([P, 181], FP32)
        cb16 = const.tile([P, 513], BF16)
        nc.sync.dma_start(cb32[:], cb32_in[:])
        nc.sync.dma_start(cb16[:], cb16_in[:])
        ones_f = cb32[:, 0:128]
        aq = cb32[:, 128:130]
        ak = cb32[:, 130:131]
        av = cb32[:, 131:132]
        ao = cb32[:, 132:148]
        ag = cb32[:, 148:156]
        au = cb32[:, 156:164]
        ad = cb32[:, 164:180]
        eps_t = cb32[:, 180:181]
        ones_b = cb16[:, 0:1]
        rT = cb16[:, 1:129]
        iden = cb16[:, 129:257]
        tril2 = cb16[:, 257:513]

        wdres = tc.alloc_tile_pool(name="wd_res", bufs=1)
        wd_sb = wdres.tile([P, IT, H], F8)
        nc.scalar.dma_start(wd_sb[:], wd_in[:])
        wgur = tc.alloc_tile_pool(name="wgu_res", bufs=1)
        wg_sb = wgur.tile([P, IT, HT, P], F8)
        wu_sb = wgur.tile([P, IT, HT, P], F8)
        for _f in range(IT):
            nc.scalar.dma_start(wg_sb[:, _f], wg_in[_f])
            nc.scalar.dma_start(wu_sb[:, _f], wu_in[_f])

        midpool = tc.alloc_tile_pool(name="midpool", bufs=1)
        x_mid = midpool.tile([P, HT, TOK], FP32)
        xopool = tc.alloc_tile_pool(name="xopool", bufs=1)
        xo = xopool.tile([P, HT, TOK], FP32)
        omypool = tc.alloc_tile_pool(name="omypool", bufs=1)
        o_my = omypool.tile([P, HT, TOK], BF16)      # post-A2A: 16 heads x my toks
        qkvpool = tc.alloc_tile_pool(name="qkvpool", bufs=1)
        q_my = qkvpool.tile([P, 2, S], BF16)         # my 2 heads, all tokens
        k_my = qkvpool.tile([P, B, P], BF16)         # my kv head [d, blk, tok]
        v_my = qkvpool.tile([P, B, P], BF16)         # my kv head [tok, blk, d]

        def rms_bcast(src3d, nt, psp, tmp):
            """[P,HT,nt] -> [P,nt] fp32 SBUF broadcast of 1/sqrt(mean_sq+eps).

            Squares on ScalarE, kt-tree-sum on GpSimd (both otherwise idle),
            one ones-mm partition-reduce, Abs_reciprocal_sqrt, PE broadcast."""
            ssq = psp.tile([1, nt], FP32, name="ssq", tag="rmsps")
            for kt in range(HT):
                sqv = tmp.tile([P, nt], BF16, name="sqv")
                nc.scalar.activation(sqv[:], src3d[:, kt, :], AF.Square)
                nc.tensor.matmul(ssq[:], ones_b, sqv[:],
                                 start=(kt == 0), stop=(kt == HT - 1))
            rsq = tmp.tile([1, nt], FP32, name="rsq")
            nc.scalar.activation(rsq[:], ssq[:], AF.Abs_reciprocal_sqrt,
                                 bias=eps_t[0:1, :], scale=1.0 / H)
            bc = psp.tile([P, nt], FP32, name="bc", tag="rmsps")
            nc.tensor.matmul(bc[:], ones_f[0:1, :], rsq[:], start=True, stop=True)
            bcs = tmp.tile([P, nt], FP32, name="bcs")
            nc.scalar.activation(bcs[:], bc[:], AF.Copy)
            return bcs

        def rmsnorm_t(src3d, out3d, nt, psp, tmp):
            """[P,HT,nt] fp32 -> bf16 rmsnorm (partition reduce via ones-mm)"""
            bcs = rms_bcast(src3d, nt, psp, tmp)
            for kt in range(HT):
                nc.vector.tensor_mul(out3d[:, kt, :], src3d[:, kt, :], bcs[:])

        # ====== phase 1: ln1 (all tokens, chunked) + q/k/v TP projections ======
        CH4 = 512
        with tc.tile_pool(name="xc_pool", bufs=2) as xcp, \
             tc.tile_pool(name="hc_pool", bufs=2) as hcp, \
             tc.tile_pool(name="p1sb", bufs=2) as p1sb, \
             tc.tile_pool(name="p1ps", bufs=1, space="PSUM") as p1ps, \
             tc.tile_pool(name="wpool", bufs=2) as wp, \
             tc.tile_pool(name="p2ps", bufs=3, space="PSUM") as p2ps, \
             tc.tile_pool(name="rot_ps", bufs=2, space="PSUM") as rot_ps, \
             tc.tile_pool(name="vt_ps", bufs=2, space="PSUM") as vt_ps, \
             tc.tile_pool(name="p2sb", bufs=2) as p2sb, \
             tc.tile_pool(name="cs_pool", bufs=2) as csp, \
             tc.tile_pool(name="wres", bufs=1) as wres:
            wq_sb = wres.tile([P, 2, HT, P], BF16)
            nc.sync.dma_start(wq_sb[:], wq_in[:])
            wk_sb = wres.tile([P, HT, P], BF16)
            nc.sync.dma_start(wk_sb[:], wk_in[:])
            wv_sb = wres.tile([P, HT, P], BF16)
            nc.sync.dma_start(wv_sb[:], wv_in[:])
            for c4 in range(4):
                tsl = slice(c4 * CH4, (c4 + 1) * CH4)
                xc = xcp.tile([P, HT, CH4], BF16, name="xc")
                nc.sync.dma_start(xc[:], xT_f[:, :, tsl])
                cfc = csp.tile([P, CH4], FP32, name="cfc")
                nc.sync.dma_start(cfc[:], cos_f[:, tsl])
                sfc = csp.tile([P, CH4], FP32, name="sfc")
                nc.sync.dma_start(sfc[:], sin_f[:, tsl])
                with nc.named_scope("p1"):
                    # rmsnorm per-token scale s_t, folded into cos/sin and the
                    # v evict (projections are linear in s_t) — no normalized
                    # h tile is ever materialized.
                    bcs = rms_bcast(xc, CH4, p1ps, p1sb)
                    cf2 = csp.tile([P, CH4], BF16, name="cf2")
                    nc.vector.tensor_mul(cf2[:], cfc[:], bcs[:])
                    sf2 = csp.tile([P, CH4], BF16, name="sf2")
                    nc.vector.tensor_mul(sf2[:], sfc[:], bcs[:])
                    # q: my 2 heads
                    for f in range(2):
                        ps = p2ps.tile([P, CH4], FP32, name="pps")
                        for kt in range(HT):
                            nc.tensor.matmul(ps[:], wq_sb[:, f, kt, :],
                                             xc[:, kt, :],
                                             start=(kt == 0), stop=(kt == HT - 1))
                        qs = p2sb.tile([P, CH4], BF16, name="qs")
                        nc.vector.tensor_scalar_mul(qs[:], ps[:], aq[:, f:f + 1])
                        rot = rot_ps.tile([P, CH4], FP32, name="rot")
                        nc.tensor.matmul(rot[:], rT[:], qs[:], start=True, stop=True)
                        t1 = p2sb.tile([P, CH4], BF16, name="t1")
                        nc.vector.tensor_mul(t1[:], rot[:], sf2[:])
                        t2 = p2sb.tile([P, CH4], BF16, name="t2")
                        nc.vector.tensor_mul(t2[:], qs[:], cf2[:])
                        nc.vector.tensor_add(q_my[:, f, tsl], t1[:], t2[:])
                    # k: my kv head
                    ps = p2ps.tile([P, CH4], FP32, name="pps")
                    for kt in range(HT):
                        nc.tensor.matmul(ps[:], wk_sb[:, kt, :], xc[:, kt, :],
                                         start=(kt == 0), stop=(kt == HT - 1))
                    ks = p2sb.tile([P, CH4], BF16, name="qs")
                    nc.vector.tensor_scalar_mul(ks[:], ps[:], ak[:, 0:1])
                    rot = rot_ps.tile([P, CH4], FP32, name="rot")
                    nc.tensor.matmul(rot[:], rT[:], ks[:], start=True, stop=True)
                    t1 = p2sb.tile([P, CH4], BF16, name="t1")
                    nc.vector.tensor_mul(t1[:], rot[:], sf2[:])
                    t2 = p2sb.tile([P, CH4], BF16, name="t2")
                    nc.vector.tensor_mul(t2[:], ks[:], cf2[:])
                    nc.vector.tensor_add(
                        k_my[:, 4 * c4:4 * c4 + 4, :].rearrange("p b t -> p (b t)"),
                        t1[:], t2[:])
                    # v: my kv head (rms scale via stt), PE-transpose to [tok, d]
                    ps = p2ps.tile([P, CH4], FP32, name="pps")
                    for kt in range(HT):
                        nc.tensor.matmul(ps[:], wv_sb[:, kt, :], xc[:, kt, :],
                                         start=(kt == 0), stop=(kt == HT - 1))
                    vtv = p2sb.tile([P, CH4], BF16, name="vtv")
                    nc.vector.scalar_tensor_tensor(
                        vtv[:], ps[:], av[:, 0:1], bcs[:], ALU.mult, ALU.mult)
                    for j in range(4):
                        vtp = vt_ps.tile([P, P], BF16, name="vtp")
                        nc.tensor.transpose(vtp[:], vtv[:, j * P:(j + 1) * P],
                                            iden[:])
                        nc.vector.tensor_copy(v_my[:, 4 * c4 + j, :], vtp[:])

        # ============= phase 2: attention (triangle, paired heads) =============
        # exp batched over groups of 4 key-blocks (one ACT op per group).
        with tc.tile_pool(name="a_ps", bufs=2, space="PSUM") as a_ps, \
             tc.tile_pool(name="o_ps", bufs=2, space="PSUM") as o_ps, \
             tc.tile_pool(name="l_ps", bufs=1, space="PSUM") as l_ps, \
             tc.tile_pool(name="bc_ps", bufs=1, space="PSUM") as bc_ps, \
             tc.tile_pool(name="a_sb", bufs=3) as a_sb:
            for qb in range(B):
                r_dst = min(qb, 15 - qb)
                ops = o_ps.tile([P, TOK], FP32, name="ops")
                lps = l_ps.tile([1, TOK], FP32, name="lps")
                qv = q_my[:, :, qb * P:(qb + 1) * P]    # [P, 2, 128]
                with nc.named_scope("attn"):
                    ngrp = (qb + 4) // 4
                    for g in range(ngrp):
                        gsz = min(4, qb + 1 - 4 * g)
                        sps = a_ps.tile([P, 4 * TOK], FP32, name="sps")
                        for j in range(gsz):
                            nc.tensor.matmul(
                                sps[:, j * TOK:(j + 1) * TOK],
                                k_my[:, 4 * g + j, :], qv,
                                start=True, stop=True)
                        pm = a_sb.tile([P, 4 * TOK], BF16, name="pm")
                        nc.scalar.activation(pm[:, :gsz * TOK],
                                             sps[:, :gsz * TOK], AF.Exp)
                        for j in range(gsz):
                            kb = 4 * g + j
                            blk = pm[:, j * TOK:(j + 1) * TOK]
                            if kb == qb:
                                pmm = a_sb.tile([P, TOK], BF16, name="pmm")
                                nc.vector.tensor_mul(pmm[:], blk, tril2[:])
                                blk = pmm[:]
                            nc.tensor.matmul(lps[:], ones_b[:], blk,
                                             start=(kb == 0), stop=(kb == qb))
                            nc.tensor.matmul(ops[:], v_my[:, kb, :], blk,
                                             start=(kb == 0), stop=(kb == qb))
                    lsb = a_sb.tile([1, TOK], FP32, name="lsb")
                    nc.scalar.activation(lsb[:], lps[:], AF.Copy)
                    linv = a_sb.tile([1, TOK], FP32, name="linv")
                    nc.vector.reciprocal_approx_fast(linv[:], lsb[:])
                    bca = bc_ps.tile([P, TOK], FP32, name="bca")
                    nc.tensor.matmul(bca[:], ones_f[0:1, :], linv[:],
                                     start=True, stop=True)
                    bcs = a_sb.tile([P, TOK], FP32, name="bcs")
                    nc.scalar.activation(bcs[:], bca[:], AF.Copy)
                    osb = a_sb.tile([P, TOK], BF16, name="osb")
                    nc.vector.tensor_mul(osb[:], ops[:], bcs[:])
                dst = a2a_lo_in if qb < 8 else a2a_hi_in
                nc.sync.dma_start(
                    dst[r_dst][:],
                    osb[:].rearrange("p (h t) -> p h t", h=2))
                if qb == 7:
                    nc.gpsimd.collective_compute(
                        "AllToAll", ALU.bypass, ins=[a2a_lo_in[:]],
                        outs=[a2a_lo_out[:]], replica_groups=rg)
                    for hh in range(2):
                        nc.sync.dma_start(
                            o_my[:].rearrange("p (j h) t -> h p j t", h=2)[hh][:, :, 0:P],
                            a2a_lo_out[:].rearrange("j p h t -> h p j t")[hh])
            nc.gpsimd.collective_compute(
                "AllToAll", ALU.bypass, ins=[a2a_hi_in[:]],
                outs=[a2a_hi_out[:]], replica_groups=rg)
            for hh in range(2):
                nc.sync.dma_start(
                    o_my[:].rearrange("p (j h) t -> h p j t", h=2)[hh][:, :, P:TOK],
                    a2a_hi_out[:].rearrange("j p h t -> h p j t")[hh])
        qkvpool.release()

        # ============= phase 3: o_proj + residual + ln2 (token halves) =============
        with tc.tile_pool(name="wo_res", bufs=1) as wores, \
             tc.tile_pool(name="p5ps", bufs=2, space="PSUM") as p5ps, \
             tc.tile_pool(name="p5sb", bufs=3) as p5sb:
            nc.sync.dma_start(xo[:], xT_own[:])
            wo_all = wores.tile([P, HT, HT, P], BF16)
            for f in range(HT):
                nc.scalar.dma_start(wo_all[:, f, :, :], wo_in[f])
            for half, (h2_in, h2_g) in enumerate(
                    ((h2_in_lo, h2_glo), (h2_in_hi, h2_ghi))):
                csl = slice(half * P, (half + 1) * P)
                with nc.named_scope("oproj"):
                    for f in range(HT):
                        ps = p5ps.tile([P, P], FP32, name="ops5")
                        for kt in range(HT):
                            nc.tensor.matmul(ps[:], wo_all[:, f, kt, :],
                                             o_my[:, kt, csl],
                                             start=(kt == 0), stop=(kt == HT - 1))
                        nc.vector.scalar_tensor_tensor(
                            x_mid[:, f, csl], ps[:], ao[:, f:f + 1],
                            xo[:, f, csl], ALU.mult, ALU.add)
                    h2h = p5sb.tile([P, HT, P], F8, name="h2h", tag="h2h")
                    rmsnorm_t(x_mid[:, :, csl], h2h, P, p5ps, p5sb)
                nc.gpsimd.dma_start(h2_in[:], h2h[:])
                nc.gpsimd.collective_compute(
                    "AllGather", ALU.bypass, ins=[h2_in[:]],
                    outs=[h2_g[:]], replica_groups=rg)
            nc.gpsimd.dma_start(xmidT[:], x_mid[:])
        omypool.release()
        xopool.release()
        midpool.release()
        h2lov = h2_glo[:].rearrange("(r p) kt t -> r p kt t", r=NC)
        h2hiv = h2_ghi[:].rearrange("(r p) kt t -> r p kt t", r=NC)

        # ============= phase 5: MLP (TP over inter) + RS =============
        CHM = 512
        with tc.tile_pool(name="h2c_pool", bufs=2) as h2cp, \
             tc.tile_pool(name="m_pool", bufs=2) as mp, \
             tc.tile_pool(name="p7gu", bufs=3, space="PSUM") as p7gu, \
             tc.tile_pool(name="p7d", bufs=2, space="PSUM") as p7d, \
             tc.tile_pool(name="p7sb", bufs=3) as p7sb:
            for c in range(4):
                h2v = h2lov if c < 2 else h2hiv
                rbase = (c % 2) * 4
                h2c = h2cp.tile([P, HT, CHM], F8, name="h2c")
                for j in range(4):
                    nc.sync.dma_start(h2c[:, :, j * P:(j + 1) * P],
                                      h2v[rbase + j])
                m_all = mp.tile([P, IT, CHM], F8, name="m_all")
                with nc.named_scope("mlp"):
                    for f in range(IT):
                        gps = p7gu.tile([P, CHM], FP32, name="gps")
                        for tp in range(HT // 2):
                            nc.tensor.matmul(
                                gps[:], wg_sb[:, f, 2 * tp:2 * tp + 2, :],
                                h2c[:, 2 * tp:2 * tp + 2, :],
                                start=(tp == 0), stop=(tp == HT // 2 - 1),
                                perf_mode=mybir.MatmulPerfMode.DoubleRow)
                        ups = p7gu.tile([P, CHM], FP32, name="ups")
                        for tp in range(HT // 2):
                            nc.tensor.matmul(
                                ups[:], wu_sb[:, f, 2 * tp:2 * tp + 2, :],
                                h2c[:, 2 * tp:2 * tp + 2, :],
                                start=(tp == 0), stop=(tp == HT // 2 - 1),
                                perf_mode=mybir.MatmulPerfMode.DoubleRow)
                        # relu(ag*g) then square on ScalarE; final mul on DVE
                        grl = p7sb.tile([P, CHM], BF16, name="grl")
                        nc.scalar.activation(grl[:], gps[:], AF.Relu,
                                             scale=ag[:, f:f + 1])
                        g2 = p7sb.tile([P, CHM], BF16, name="g2")
                        nc.scalar.activation(g2[:], grl[:], AF.Square)
                        nc.vector.scalar_tensor_tensor(m_all[:, f, :], ups[:],
                                                       au[:, f:f + 1], g2[:],
                                                       ALU.mult, ALU.mult)
                rs_a = nc.dram_tensor(f"rs_in_{c}a", [H, TOK], BF16)
                rs_b = nc.dram_tensor(f"rs_in_{c}b", [H, TOK], BF16)
                rs_iva = rs_a[:].rearrange("(f p) t -> f p t", p=P)
                rs_ivb = rs_b[:].rearrange("(f p) t -> f p t", p=P)
                if c < 3:
                    for f in range(HT):
                        dps = p7d.tile([P, CHM], FP32, name="dps")
                        for tp in range(IT // 2):
                            nc.tensor.matmul(
                                dps[:], wd_sb[:, 2 * tp:2 * tp + 2, f * P:(f + 1) * P],
                                m_all[:, 2 * tp:2 * tp + 2, :],
                                start=(tp == 0), stop=(tp == IT // 2 - 1),
                                perf_mode=mybir.MatmulPerfMode.DoubleRow)
                        dn = p7sb.tile([P, CHM], BF16, name="dn")
                        nc.vector.tensor_scalar_mul(dn[:], dps[:], ad[:, f:f + 1])
                        nc.sync.dma_start(rs_iva[f], dn[:, 0:TOK])
                        nc.sync.dma_start(rs_ivb[f], dn[:, TOK:CHM])
                    for hf, rs_in in enumerate((rs_a, rs_b)):
                        rs_out = nc.dram_tensor(f"rso_{c}{hf}", [TOK, TOK], BF16)
                        nc.gpsimd.collective_compute(
                            "ReduceScatter", ALU.add, ins=[rs_in[:]],
                            outs=[rs_out[:]], replica_groups=rg)
                        nc.sync.dma_start(
                            outT[:, (2 * c + hf) * TOK:(2 * c + hf + 1) * TOK],
                            rs_out[:])
                else:
                    # last chunk: split down by token halves so RS pieces overlap
                    for hf, (rs_in, rs_iv) in enumerate(
                            ((rs_a, rs_iva), (rs_b, rs_ivb))):
                        tsl2 = slice(hf * TOK, (hf + 1) * TOK)
                        for f in range(HT):
                            dps = p7d.tile([P, CHM], FP32, name="dps")[:, 0:TOK]
                            for tp in range(IT // 2):
                                nc.tensor.matmul(
                                    dps[:],
                                    wd_sb[:, 2 * tp:2 * tp + 2, f * P:(f + 1) * P],
                                    m_all[:, 2 * tp:2 * tp + 2, tsl2],
                                    start=(tp == 0), stop=(tp == IT // 2 - 1),
                                    perf_mode=mybir.MatmulPerfMode.DoubleRow)
                            dn = p7sb.tile([P, CHM], BF16, name="dn")[:, 0:TOK]
                            nc.vector.tensor_scalar_mul(dn[:], dps[:],
                                                        ad[:, f:f + 1])
                            nc.sync.dma_start(rs_iv[f], dn[:])
                        rs_out = nc.dram_tensor(f"rso_3{hf}", [TOK, TOK], BF16)
                        nc.gpsimd.collective_compute(
                            "ReduceScatter", ALU.add, ins=[rs_in[:]],
                            outs=[rs_out[:]], replica_groups=rg)
                        nc.sync.dma_start(
                            outT[:, (6 + hf) * TOK:(7 + hf) * TOK], rs_out[:])

        wgur.release()
        wdres.release()
        const.release()

    nc.finalize()
    return nc


def _ternary(w, fold_row=None):
    """Quantize [O, Hin] fp32 -> (ternary fp32 {-1,0,1}, absmean [O])."""
    w = np.asarray(w, dtype=np.float32)
    am = np.mean(np.abs(w), axis=1)
    t = np.sign(w) * (np.abs(w) > ALPHA * am[:, None]).astype(np.float32)
    if fold_row is not None:
        t = t * fold_row[None, :]
    return t, am


def _wlhsT(tern, n_f):
    """ternary [O, Hin] -> lhsT input layout [f, p, kt, c] bf16 (tile (kt,f):
    rows Hin-chunk kt, cols O-chunk f)."""
    o, hin = tern.shape
    kt = hin // P
    assert n_f * P == o
    wT = np.ascontiguousarray(tern.T)  # [Hin, O]
    return np.ascontiguousarray(
        wT.reshape(kt, P, n_f, P).transpose(2, 1, 0, 3)).astype(BF)


def _scale_tiles(a):
    """[O] -> [P, O//P] with column f = features f*128..f*128+127."""
    return np.ascontiguousarray(a.reshape(-1, P).T).astype(np.float32)


def _pcol(x2d):
    """[K, T] -> [P, K//P, T] (partition-major for direct DMA)."""
    k, t = x2d.shape
    return np.ascontiguousarray(
        x2d.reshape(k // P, P, t).transpose(1, 0, 2)).astype(np.float32)


def kernel(x, cos, sin, wq, wk, wv, wo, wg, wu, wd, ln1_w, ln2_w):
    x = np.asarray(x, dtype=np.float32)
    b, s, hdim = x.shape
    assert (b, s, hdim) == (1, S, H)

    if "nc" not in _CACHE:
        _CACHE["nc"] = _build_program()
    nc = _CACHE["nc"]

    ln1 = np.asarray(ln1_w, dtype=np.float32)
    ln2 = np.asarray(ln2_w, dtype=np.float32)

    tq, amq = _ternary(wq, fold_row=ln1)
    tk, amk = _ternary(wk, fold_row=ln1)
    tv, amv = _ternary(wv, fold_row=ln1)
    to, amo = _ternary(wo)
    tg, amg = _ternary(wg, fold_row=ln2)
    tu, amu = _ternary(wu, fold_row=ln2)
    td, amd = _ternary(wd)

    wq_h = _wlhsT(tq, NH)        # [16, P, HT, P]
    wk_h = _wlhsT(tk, NKV)       # [4, P, HT, P]
    wv_h = _wlhsT(tv, NKV)
    wo_h = _wlhsT(to, HT)
    wg_h = _wlhsT(tg, I_TOT // P)
    wu_h = _wlhsT(tu, I_TOT // P)
    wd_h = np.ascontiguousarray(
        td.T.reshape(I_TOT // P, P, H).transpose(1, 0, 2)).astype(BF)  # [P,64,H]

    aq_h = _scale_tiles(amq / np.sqrt(np.float32(D)))
    ak_h = _scale_tiles(amk)
    av_h = _scale_tiles(amv)
    ao_h = _scale_tiles(amo)
    ag_h = _scale_tiles(amg)
    au_h = _scale_tiles(amu)
    ad_h = _scale_tiles(amd)

    x2 = x[0]
    xT = np.ascontiguousarray(x2.T)
    xT_f = _pcol(xT)
    cosT = np.ascontiguousarray(np.asarray(cos, np.float32)[0, 0].T)
    sinT = np.ascontiguousarray(np.asarray(sin, np.float32)[0, 0].T)

    R = np.zeros((P, P), np.float32)
    for m in range(64):
        R[m, m + 64] = -1.0
        R[m + 64, m] = 1.0
    rT_h = np.ascontiguousarray(R.T).astype(BF)
    ones_f = np.ones((P, P), np.float32)
    ones_b = np.ones((P, 1), np.float32).astype(BF)
    triu = np.triu(np.ones((P, P), np.float32))
    tril2_h = np.ascontiguousarray(np.concatenate([triu, triu], axis=1)).astype(BF)
    iden_h = np.eye(P, dtype=np.float32).astype(BF)

    in_maps = []
    for i in range(NC):
        blo, bhi = i, 15 - i
        own_cols = np.r_[blo * P:(blo + 1) * P, bhi * P:(bhi + 1) * P]
        kvh = i // 2
        islice = slice(i * IT, (i + 1) * IT)
        cb32 = np.ascontiguousarray(np.concatenate([
            ones_f,
            aq_h[:, 2 * i:2 * i + 2],
            ak_h[:, kvh:kvh + 1],
            av_h[:, kvh:kvh + 1],
            ao_h,
            ag_h[:, islice],
            au_h[:, islice],
            ad_h,
            np.full((P, 1), EPS, np.float32),
        ], axis=1)).astype(np.float32)
        cb16 = np.ascontiguousarray(np.concatenate([
            ones_b.astype(np.float32), rT_h.astype(np.float32),
            iden_h.astype(np.float32), tril2_h.astype(np.float32),
        ], axis=1)).astype(BF)
        in_maps.append({
            "xT_f": xT_f.astype(BF),
            "xT_own": _pcol(xT[:, own_cols]),
            "cos_f": cosT, "sin_f": sinT,
            "wq": np.ascontiguousarray(wq_h[2 * i:2 * i + 2].transpose(1, 0, 2, 3)),
            "wk": np.ascontiguousarray(wk_h[kvh]),
            "wv": np.ascontiguousarray(wv_h[kvh]),
            "wo": wo_h,
            "wg": np.ascontiguousarray(wg_h[islice]).astype(E4),
            "wu": np.ascontiguousarray(wu_h[islice]).astype(E4),
            "wd": np.ascontiguousarray(wd_h[:, islice, :]).astype(E4),
            "cb32": cb32, "cb16": cb16,
        })

    res = run_bass_kernel_spmd(nc, in_maps, list(range(NC)))
    _CACHE["last_result"] = res

    down_T = np.concatenate(
        [res.results[i]["outT"].astype(np.float32) for i in range(NC)], axis=0)
    xmid_T = np.concatenate(
        [res.results[i]["xmidT"].transpose(1, 0, 2).reshape(H, TOK)
         for i in range(NC)], axis=1)
    out_T = np.empty_like(down_T)
    for j, blk in enumerate(PERM_DOWN):
        out_T[:, blk * P:(blk + 1) * P] = down_T[:, j * P:(j + 1) * P]
    for j, blk in enumerate(PERM):
        out_T[:, blk * P:(blk + 1) * P] += xmid_T[:, j * P:(j + 1) * P]
    return np.ascontiguousarray(out_T.T).reshape(1, S, H).astype(np.float32)


if __name__ == "__main__":
    nc = _build_program()
    print("build OK; instructions:",
          sum(len(b.instructions) for f in nc.m.functions for b in f.blocks))

